# revision 25
# baseline (speedup 1.0000x reference)
"""Trainium2 Bass kernel for a 2-layer input-feed LSTM decoder stack.

Shapes: S=128 steps, B=64 batch, V=32000 vocab, E=512 embed, H=1024 hidden.

Strategy (8 NeuronCores):
  - Shard the hidden/gate dimension 8-way: core j owns H-slice
    [128j:128j+128) of both layers, i.e. 512 of the 4096 gate rows
    (order [i, f, o, g] within the core for ACT batching).
  - All weight slices live SBUF-resident for the whole kernel (~9.4 MB/core
    fp32) -> no HBM weight streaming inside the time loop.
  - Matmuls are activation-stationary: lhsT = x^T / h^T tiles [128, 64],
    rhs = W^T blocks [128, 512] streaming through the PE.
  - Gates accumulate in one PSUM bank [64, 512]; biases are added by a
    K=1 matmul against a ones row; the embedding contribution uses
    host-pregathered emb^T slices (off the critical path).
  - After each layer's elementwise, the core's h^T chunk [128, 64] is
    all-gathered across the 8 cores (collective AllGather via DRAM bounce)
    to form the full h^T [128, 512] needed by the next matmuls.

The host (this file) does the embedding gather, the per-core weight
slicing/transposition/swizzling, runs the SPMD kernel on cores 0-7 via
run_bass_kernel_spmd, and reassembles full outputs.
"""

import sys

sys.path.insert(0, "/opt/trn_rl_repo")

import numpy as np

import concourse.bass as bass
import concourse.bacc as bacc
import concourse.mybir as mybir
import concourse.tile as tile
from concourse.bass_utils import run_bass_kernel_spmd
from concourse.tile_rust import add_dep_helper

S, B, V, E, H = 128, 64, 32000, 512, 1024
NC = 8  # cores
HC = H // NC  # 128, per-core H slice
G = 4 * HC  # 512, per-core gate rows
KB_H = H // 128  # 8 K-blocks for an H-wide contraction
KB_E = E // 128  # 4 K-blocks for the embedding contraction

FP = mybir.dt.float32
AF = mybir.ActivationFunctionType


def _build_nc(s_steps: int, comm: str = "collective"):
    """Build the SPMD Bass program (same program on all 8 cores).

    comm="collective": all-gather h^T chunks via collective_compute (DRAM
        bounce, ~5-7us latency each).
    comm="remote": one-hop mesh all-gather via remote_dma_broadcast directly
        between SBUFs (XOR slot layout: slot p on core j holds the chunk of
        core j^p; the host permutes weight K-blocks to match).
    """
    nc = bacc.Bacc(
        "TRN2",
        target_bir_lowering=False,
        debug=False,
        num_devices=NC,
        num_swdge_queues=2,
    )
    # (instruction, sem, value) cross-core waits, attached AFTER Tile
    # scheduling (the single-core scheduling sim can't see remote sem
    # increments and would deadlock on them).
    deferred_waits: list = []

    # ---- DRAM I/O ----
    embT_d = nc.dram_tensor("embT", [s_steps, 128, KB_E * B], FP, kind="ExternalInput")
    w_e0_d = nc.dram_tensor("w_e0", [128, KB_E * G], FP, kind="ExternalInput")
    w_o0_d = nc.dram_tensor("w_o0", [128, KB_H * G], FP, kind="ExternalInput")
    w_h0_d = nc.dram_tensor("w_h0", [128, KB_H * G], FP, kind="ExternalInput")
    w_i1_d = nc.dram_tensor("w_i1", [128, KB_H * G], FP, kind="ExternalInput")
    w_h1_d = nc.dram_tensor("w_h1", [128, KB_H * G], FP, kind="ExternalInput")
    bias0_d = nc.dram_tensor("bias0", [1, G], FP, kind="ExternalInput")
    bias1_d = nc.dram_tensor("bias1", [1, G], FP, kind="ExternalInput")
    ones_d = nc.dram_tensor("ones", [1, B], FP, kind="ExternalInput")
    ident_d = nc.dram_tensor("ident", [B, B], FP, kind="ExternalInput")
    outTi_d = nc.dram_tensor("outT_init", [128, NC * B], FP, kind="ExternalInput")
    h0Ti_d = nc.dram_tensor("h0T_init", [128, NC * B], FP, kind="ExternalInput")
    h1Ti_d = nc.dram_tensor("h1T_init", [128, NC * B], FP, kind="ExternalInput")
    c0i_d = nc.dram_tensor("c0_init", [B, HC], FP, kind="ExternalInput")
    c1i_d = nc.dram_tensor("c1_init", [B, HC], FP, kind="ExternalInput")

    outs_d = nc.dram_tensor("outs", [s_steps, B, HC], FP, kind="ExternalOutput")
    hn_d = nc.dram_tensor("hn", [2, B, HC], FP, kind="ExternalOutput")
    cn_d = nc.dram_tensor("cn", [2, B, HC], FP, kind="ExternalOutput")

    with tile.TileContext(nc) as tc:
        with (
            tc.tile_pool(name="wpool", bufs=1) as wpool,
            tc.tile_pool(name="embp", bufs=3) as embp,
            tc.tile_pool(name="gp", bufs=2) as gp,
            tc.tile_pool(name="sp", bufs=2) as sp,
            tc.tile_pool(name="psg", bufs=2, space="PSUM") as psg,
            tc.tile_pool(name="pst", bufs=2, space="PSUM") as pst,
            tc.tile_pool(name="dramp", bufs=2, space="DRAM") as dramp,
        ):
            # ---- load constants / weights into SBUF ----
            w_e0 = wpool.tile([128, KB_E * G], FP, tag="w_e0")
            w_o0 = wpool.tile([128, KB_H * G], FP, tag="w_o0")
            w_h0 = wpool.tile([128, KB_H * G], FP, tag="w_h0")
            w_i1 = wpool.tile([128, KB_H * G], FP, tag="w_i1")
            w_h1 = wpool.tile([128, KB_H * G], FP, tag="w_h1")
            bias0 = wpool.tile([1, G], FP, tag="bias0")
            bias1 = wpool.tile([1, G], FP, tag="bias1")
            ones = wpool.tile([1, B], FP, tag="ones")
            ident = wpool.tile([B, B], FP, tag="ident")
            outT_init = wpool.tile([128, NC * B], FP, tag="outTi")
            h0T_init = wpool.tile([128, NC * B], FP, tag="h0Ti")
            h1T_init = wpool.tile([128, NC * B], FP, tag="h1Ti")

            for sb, dr in [
                (w_e0, w_e0_d), (w_o0, w_o0_d), (w_h0, w_h0_d),
                (w_i1, w_i1_d), (w_h1, w_h1_d),
                (bias0, bias0_d), (bias1, bias1_d),
                (ones, ones_d), (ident, ident_d),
                (outT_init, outTi_d), (h0T_init, h0Ti_d), (h1T_init, h1Ti_d),
            ]:
                nc.sync.dma_start(sb[:], dr[:])

            c0_prev = sp.tile([B, HC], FP, tag="c0")
            c1_prev = sp.tile([B, HC], FP, tag="c1")
            nc.sync.dma_start(c0_prev[:], c0i_d[:])
            nc.sync.dma_start(c1_prev[:], c1i_d[:])

            outT_prev = outT_init  # gathered out^T   [128, 8*64]
            h0T_prev = h0T_init    # gathered h0^T    [128, 8*64]
            h1T_prev = h1T_init    # gathered h1^T    [128, 8*64]

            if comm == "remote":
                rsems = [nc.alloc_semaphore("rsem0"), nc.alloc_semaphore("rsem1")]
                lsems = [nc.alloc_semaphore("lsem0"), nc.alloc_semaphore("lsem1")]
                # remote_sem inc per sender per gather: 16 // 8 dests = 2
                RS_INC_PER_STEP = 2 * (NC - 1)  # 14

            def gather_collective(pt, tagp, si):
                """All-gather this core's h^T chunk [128,64] -> [128, 8*64]."""
                hbt = sp.tile([HC, B], FP, tag=f"hbt{tagp}")
                nc.vector.tensor_copy(hbt[:], pt[:])
                agin = dramp.tile([128, B], FP, tag=f"agin{tagp}")
                agout = dramp.tile([128 * NC, B], FP, tag=f"agout{tagp}")
                nc.sync.dma_start(agin[:], hbt[:])
                nc.gpsimd.collective_compute(
                    "AllGather",
                    mybir.AluOpType.bypass,
                    ins=[agin.opt()],
                    outs=[agout.opt()],
                    replica_groups=[list(range(NC))],
                )
                gt = gp.tile([128, NC * B], FP, tag=f"g{tagp}")
                nc.sync.dma_start(
                    gt[:].rearrange("p (b n) -> p b n", n=B),
                    agout[:].rearrange("(b p) n -> p b n", p=128),
                )
                return gt, None

            def gather_remote(pt, tagp, si):
                """One-hop mesh all-gather: write own chunk into slot 0, then
                remote-DMA it into slot d of each XOR peer (me^d)."""
                gt = gp.tile([128, NC * B], FP, tag=f"g{tagp}")
                cp = nc.vector.tensor_copy(gt[:, 0:B], pt[:])
                for d in range(1, NC):
                    rdests: list = [None] * NC
                    rdests[d] = (0, d)
                    nc.gpsimd.remote_dma_broadcast(
                        out_ap=gt[:, d * B : (d + 1) * B],
                        in_ap=gt[:, 0:B],
                        remote_sem=rsems[si],
                        local_sem=lsems[si],
                        rdests=rdests,
                        queue_num=si,
                    )
                nc.gpsimd.trigger_dma(count=None, queue_num=si)
                return gt, cp

            gather = gather_remote if comm == "remote" else gather_collective
            g0_cp = g1_cp = None

            def lstm_elem(pg, c_prev, tag):
                """Gate nonlinearities + cell update. pg: PSUM [64, 512] in
                [i|f|o|g] order. Returns (h_chunk [64,128], c_new [64,128])."""
                sg = sp.tile([B, 3 * HC], FP, tag=f"sg{tag}")
                tg = sp.tile([B, HC], FP, tag=f"tg{tag}")
                nc.scalar.activation(sg[:], pg[:, 0 : 3 * HC], AF.Sigmoid)
                nc.scalar.activation(tg[:], pg[:, 3 * HC : 4 * HC], AF.Tanh)
                m1 = sp.tile([B, HC], FP, tag=f"m1{tag}")
                m2 = sp.tile([B, HC], FP, tag=f"m2{tag}")
                c_new = sp.tile([B, HC], FP, tag=f"c{tag}")
                nc.vector.tensor_mul(m1[:], sg[:, HC : 2 * HC], c_prev[:])
                nc.vector.tensor_mul(m2[:], sg[:, 0:HC], tg[:])
                nc.vector.tensor_add(c_new[:], m1[:], m2[:])
                tc_ = sp.tile([B, HC], FP, tag=f"tc{tag}")
                nc.scalar.activation(tc_[:], c_new[:], AF.Tanh)
                h_chunk = sp.tile([B, HC], FP, tag=f"h{tag}")
                nc.vector.tensor_mul(h_chunk[:], sg[:, 2 * HC : 3 * HC], tc_[:])
                return h_chunk, c_new

            def transpose_chunk(h_chunk, tag):
                """[64,128] batch-major -> [128,64] feature-major (PSUM)."""
                pt = pst.tile([HC, B], FP, tag="pt")  # shared slots: 2 banks
                nc.tensor.transpose(pt[:], h_chunk[:], ident[:])
                return pt

            for t in range(s_steps):
                # ---------- layer 0 gates: [64, 512] in PSUM ----------
                pg0 = psg.tile([B, G], FP, tag="pg0")
                embt = embp.tile([128, KB_E * B], FP, tag="embt")
                nc.sync.dma_start(embt[:], embT_d[t, :, :])
                for k in range(KB_E):  # embedding part (prefetchable)
                    nc.tensor.matmul(
                        pg0[:],
                        embt[:, k * B : (k + 1) * B],
                        w_e0[:, k * G : (k + 1) * G],
                        start=(k == 0),
                        stop=False,
                    )
                nc.tensor.matmul(  # + bias0 (ones ⊗ bias row)
                    pg0[:], ones[:1, :], bias0[:1, :], start=False, stop=False
                )
                for k in range(KB_H):  # recurrent part (h0_{t-1})
                    nc.tensor.matmul(
                        pg0[:],
                        h0T_prev[:, k * B : (k + 1) * B],
                        w_h0[:, k * G : (k + 1) * G],
                        start=False,
                        stop=False,
                    )
                if comm == "remote" and t > 0:
                    # out_{t-1} = h1_{t-1}: wait for step t-1's h1 gather.
                    # Anchor on our own slot-0 copy so the scheduler places
                    # the wait right before the consuming matmuls.
                    wn = nc.tensor.nop(nofuse=True, hint="wait_g1")
                    add_dep_helper(wn.ins, g1_cp.ins, True, "g1 wait anchor")
                    deferred_waits.append((wn, rsems[1], RS_INC_PER_STEP * t))
                for k in range(KB_H):  # input-feed part (out_{t-1})
                    nc.tensor.matmul(
                        pg0[:],
                        outT_prev[:, k * B : (k + 1) * B],
                        w_o0[:, k * G : (k + 1) * G],
                        start=False,
                        stop=(k == KB_H - 1),
                    )

                h0_chunk, c0_new = lstm_elem(pg0, c0_prev, "0")
                g0, g0_cp = gather(transpose_chunk(h0_chunk, "0"), "0", 0)

                # ---------- layer 1 gates ----------
                pg1 = psg.tile([B, G], FP, tag="pg1")
                nc.tensor.matmul(  # bias1 first (starts accumulation)
                    pg1[:], ones[:1, :], bias1[:1, :], start=True, stop=False
                )
                for k in range(KB_H):  # recurrent part (h1_{t-1})
                    nc.tensor.matmul(
                        pg1[:],
                        h1T_prev[:, k * B : (k + 1) * B],
                        w_h1[:, k * G : (k + 1) * G],
                        start=False,
                        stop=False,
                    )
                if comm == "remote":
                    wn = nc.tensor.nop(nofuse=True, hint="wait_g0")
                    add_dep_helper(wn.ins, g0_cp.ins, True, "g0 wait anchor")
                    deferred_waits.append((wn, rsems[0], RS_INC_PER_STEP * (t + 1)))
                for k in range(KB_H):  # input part (h0_t, just gathered)
                    nc.tensor.matmul(
                        pg1[:],
                        g0[:, k * B : (k + 1) * B],
                        w_i1[:, k * G : (k + 1) * G],
                        start=False,
                        stop=(k == KB_H - 1),
                    )

                h1_chunk, c1_new = lstm_elem(pg1, c1_prev, "1")
                nc.sync.dma_start(outs_d[t, :, :], h1_chunk[:])
                g1, g1_cp = gather(transpose_chunk(h1_chunk, "1"), "1", 1)

                # rotate state
                h0T_prev = g0
                h1T_prev = g1
                outT_prev = g1
                c0_prev = c0_new
                c1_prev = c1_new

                if t == s_steps - 1:
                    nc.sync.dma_start(hn_d[0, :, :], h0_chunk[:])
                    nc.sync.dma_start(hn_d[1, :, :], h1_chunk[:])
                    nc.sync.dma_start(cn_d[0, :, :], c0_new[:])
                    nc.sync.dma_start(cn_d[1, :, :], c1_new[:])

    for mm, sem, val in deferred_waits:
        mm._wait_ge(sem, val)
    nc.compile()
    return nc


# ---------------------------------------------------------------------------
# Host-side data prep
# ---------------------------------------------------------------------------

def _gate_rows(j: int) -> np.ndarray:
    """Rows of the [4H, *] weight matrices owned by core j, in [i,f,o,g] order."""
    base = np.arange(j * HC, (j + 1) * HC)
    return np.concatenate([base, H + base, 3 * H + base, 2 * H + base])


def _swizzle_kblocks(wt: np.ndarray) -> np.ndarray:
    """[K, G] (K = n*128) -> [128, n*G] with K-block b at free cols [b*G:(b+1)*G]."""
    k, g = wt.shape
    nb = k // 128
    return np.ascontiguousarray(
        wt.reshape(nb, 128, g).transpose(1, 0, 2).reshape(128, nb * g)
    )


def _gatherT(full: np.ndarray, perm) -> np.ndarray:
    """[B, H] state -> gathered-transposed [128, 8*64]: slot p = chunk perm[p]^T."""
    out = np.empty((128, NC * B), dtype=np.float32)
    for p in range(NC):
        blk = full[:, perm[p] * HC : (perm[p] + 1) * HC]  # [B, HC]
        out[:, p * B : (p + 1) * B] = blk.T
    return np.ascontiguousarray(out)


def _prep_inputs(inputs: dict, s_steps: int, comm: str = "collective"):
    """Build per-core in_maps.

    K-block permutation: collective gathers land in natural rank order
    (perm[j][p] = p); the remote_dma mesh lands in XOR slot order
    (slot p on core j = chunk of core j^p), so weights/init states are
    permuted to match."""
    tgt = np.asarray(inputs["tgt_input"])[:s_steps]
    emb_W = np.asarray(inputs["emb_W"], dtype=np.float32)
    h0 = np.asarray(inputs["h0"], dtype=np.float32)
    c0 = np.asarray(inputs["c0"], dtype=np.float32)
    init_out = np.asarray(inputs["init_output"], dtype=np.float32)
    W_ih0 = np.asarray(inputs["W_ih0"], dtype=np.float32)
    W_hh0 = np.asarray(inputs["W_hh0"], dtype=np.float32)
    W_ih1 = np.asarray(inputs["W_ih1"], dtype=np.float32)
    W_hh1 = np.asarray(inputs["W_hh1"], dtype=np.float32)
    b0 = np.asarray(inputs["b_ih0"], dtype=np.float32) + np.asarray(
        inputs["b_hh0"], dtype=np.float32
    )
    b1 = np.asarray(inputs["b_ih1"], dtype=np.float32) + np.asarray(
        inputs["b_hh1"], dtype=np.float32
    )

    # embedding gather + transpose + K-block swizzle: [S, 128, 4*64]
    embs = emb_W[tgt]  # [S, B, E]
    embT = np.ascontiguousarray(
        embs.transpose(0, 2, 1)  # [S, E, B]
        .reshape(s_steps, KB_E, 128, B)
        .transpose(0, 2, 1, 3)
        .reshape(s_steps, 128, KB_E * B)
    ).astype(np.float32)

    ones = np.ones((1, B), dtype=np.float32)
    ident = np.eye(B, dtype=np.float32)

    in_maps = []
    for j in range(NC):
        rows = _gate_rows(j)
        if comm == "remote":
            perm = [j ^ p for p in range(NC)]
        else:
            perm = list(range(NC))
        hperm = np.concatenate(
            [np.arange(perm[p] * HC, (perm[p] + 1) * HC) for p in range(NC)]
        )
        w_e0 = _swizzle_kblocks(W_ih0[rows, :E].T)  # [128, 4*512]
        w_o0 = _swizzle_kblocks(W_ih0[rows, E:].T[hperm])  # [128, 8*512]
        w_h0 = _swizzle_kblocks(W_hh0[rows].T[hperm])
        w_i1 = _swizzle_kblocks(W_ih1[rows].T[hperm])
        w_h1 = _swizzle_kblocks(W_hh1[rows].T[hperm])
        in_maps.append(
            {
                "embT": embT,
                "w_e0": w_e0,
                "w_o0": w_o0,
                "w_h0": w_h0,
                "w_i1": w_i1,
                "w_h1": w_h1,
                "bias0": b0[rows][None, :].astype(np.float32),
                "bias1": b1[rows][None, :].astype(np.float32),
                "ones": ones,
                "ident": ident,
                "outT_init": _gatherT(init_out, perm),
                "h0T_init": _gatherT(h0[0], perm),
                "h1T_init": _gatherT(h0[1], perm),
                "c0_init": np.ascontiguousarray(c0[0][:, j * HC : (j + 1) * HC]),
                "c1_init": np.ascontiguousarray(c0[1][:, j * HC : (j + 1) * HC]),
            }
        )
    return in_maps


def _assemble(results, s_steps: int):
    outs = [np.asarray(r["outs"]).reshape(s_steps, B, HC) for r in results]
    hns = [np.asarray(r["hn"]).reshape(2, B, HC) for r in results]
    cns = [np.asarray(r["cn"]).reshape(2, B, HC) for r in results]
    outputs = np.concatenate(outs, axis=2)  # [S, B, H]
    h_n = np.concatenate(hns, axis=2)  # [2, B, H]
    c_n = np.concatenate(cns, axis=2)  # [2, B, H]
    return outputs, h_n, c_n


_NC_CACHE: dict = {}

COMM = "collective"  # module-level default comm mode


def _get_nc(s_steps: int, comm: str):
    key = (s_steps, comm)
    if key not in _NC_CACHE:
        _NC_CACHE[key] = _build_nc(s_steps, comm)
    return _NC_CACHE[key]


def kernel(
    tgt_input,
    h0,
    c0,
    init_output,
    emb_W,
    W_ih0,
    W_hh0,
    b_ih0,
    b_hh0,
    W_ih1,
    W_hh1,
    b_ih1,
    b_hh1,
    _s_steps: int = S,
    _trace: bool = False,
):
    inputs = dict(
        tgt_input=tgt_input,
        h0=h0,
        c0=c0,
        init_output=init_output,
        emb_W=emb_W,
        W_ih0=W_ih0,
        W_hh0=W_hh0,
        b_ih0=b_ih0,
        b_hh0=b_hh0,
        W_ih1=W_ih1,
        W_hh1=W_hh1,
        b_ih1=b_ih1,
        b_hh1=b_hh1,
    )
    nc = _get_nc(_s_steps, COMM)
    in_maps = _prep_inputs(inputs, _s_steps, COMM)
    res = run_bass_kernel_spmd(
        nc, in_maps, core_ids=list(range(NC)), trace=_trace
    )
    out = _assemble(res.results, _s_steps)
    if _trace:
        return out, res
    return out


# revision 34
# speedup vs baseline: 1.2083x; 1.2083x over previous
"""Trainium2 Bass kernel for a 2-layer input-feed LSTM decoder stack.

Shapes: S=128 steps, B=64 batch, V=32000 vocab, E=512 embed, H=1024 hidden.

Strategy (8 NeuronCores):
  - Shard the hidden/gate dimension 8-way: core j owns H-slice
    [128j:128j+128) of both layers, i.e. 512 of the 4096 gate rows
    (order [i, f, o, g] within the core for ACT batching).
  - All weight slices live SBUF-resident for the whole kernel (~9.4 MB/core
    fp32) -> no HBM weight streaming inside the time loop.
  - Matmuls are activation-stationary: lhsT = x^T / h^T tiles [128, 64],
    rhs = W^T blocks [128, 512] streaming through the PE.
  - Gates accumulate in one PSUM bank [64, 512]; biases are added by a
    K=1 matmul against a ones row; the embedding contribution uses
    host-pregathered emb^T slices (off the critical path).
  - After each layer's elementwise, the core's h^T chunk [128, 64] is
    all-gathered across the 8 cores (collective AllGather via DRAM bounce)
    to form the full h^T [128, 512] needed by the next matmuls.

The host (this file) does the embedding gather, the per-core weight
slicing/transposition/swizzling, runs the SPMD kernel on cores 0-7 via
run_bass_kernel_spmd, and reassembles full outputs.
"""

import sys

sys.path.insert(0, "/opt/trn_rl_repo")

import numpy as np

import concourse.bass as bass
import concourse.bacc as bacc
import concourse.mybir as mybir
import concourse.tile as tile
from concourse.bass_utils import run_bass_kernel_spmd
from concourse.tile_rust import add_dep_helper

S, B, V, E, H = 128, 64, 32000, 512, 1024
NC = 8  # cores
HC = H // NC  # 128, per-core H slice
G = 4 * HC  # 512, per-core gate rows
KB_H = H // 128  # 8 K-blocks for an H-wide contraction
KB_E = E // 128  # 4 K-blocks for the embedding contraction

FP = mybir.dt.float32
AF = mybir.ActivationFunctionType


def _build_nc(s_steps: int, comm: str = "collective"):
    """Build the SPMD Bass program (same program on all 8 cores).

    comm="collective": all-gather h^T chunks via collective_compute (DRAM
        bounce, ~5-7us latency each).
    comm="remote": one-hop mesh all-gather via remote_dma_broadcast directly
        between SBUFs (XOR slot layout: slot p on core j holds the chunk of
        core j^p; the host permutes weight K-blocks to match).
    """
    nc = bacc.Bacc(
        "TRN2",
        target_bir_lowering=False,
        debug=False,
        num_devices=NC,
        num_swdge_queues=2,
    )
    # (instruction, sem, value) cross-core waits, attached AFTER Tile
    # scheduling (the single-core scheduling sim can't see remote sem
    # increments and would deadlock on them).
    deferred_waits: list = []

    # ---- DRAM I/O ----
    embT_d = nc.dram_tensor("embT", [s_steps, 128, KB_E * B], FP, kind="ExternalInput")
    w_e0_d = nc.dram_tensor("w_e0", [128, KB_E * G], FP, kind="ExternalInput")
    w_o0_d = nc.dram_tensor("w_o0", [128, KB_H * G], FP, kind="ExternalInput")
    w_h0_d = nc.dram_tensor("w_h0", [128, KB_H * G], FP, kind="ExternalInput")
    w_i1_d = nc.dram_tensor("w_i1", [128, KB_H * G], FP, kind="ExternalInput")
    w_h1_d = nc.dram_tensor("w_h1", [128, KB_H * G], FP, kind="ExternalInput")
    bias0_d = nc.dram_tensor("bias0", [1, G], FP, kind="ExternalInput")
    bias1_d = nc.dram_tensor("bias1", [1, G], FP, kind="ExternalInput")
    ones_d = nc.dram_tensor("ones", [1, B], FP, kind="ExternalInput")
    ident_d = nc.dram_tensor("ident", [B, B], FP, kind="ExternalInput")
    outTi_d = nc.dram_tensor("outT_init", [128, NC * B], FP, kind="ExternalInput")
    h0Ti_d = nc.dram_tensor("h0T_init", [128, NC * B], FP, kind="ExternalInput")
    h1Ti_d = nc.dram_tensor("h1T_init", [128, NC * B], FP, kind="ExternalInput")
    c0i_d = nc.dram_tensor("c0_init", [B, HC], FP, kind="ExternalInput")
    c1i_d = nc.dram_tensor("c1_init", [B, HC], FP, kind="ExternalInput")

    outs_d = nc.dram_tensor("outs", [s_steps, B, HC], FP, kind="ExternalOutput")
    hn_d = nc.dram_tensor("hn", [2, B, HC], FP, kind="ExternalOutput")
    cn_d = nc.dram_tensor("cn", [2, B, HC], FP, kind="ExternalOutput")

    with tile.TileContext(nc) as tc:
        with (
            tc.tile_pool(name="wpool", bufs=1) as wpool,
            tc.tile_pool(name="embp", bufs=3) as embp,
            tc.tile_pool(name="gp", bufs=2) as gp,
            tc.tile_pool(name="sp", bufs=2) as sp,
            tc.tile_pool(name="psg", bufs=2, space="PSUM") as psg,
            tc.tile_pool(name="pst", bufs=2, space="PSUM") as pst,
            tc.tile_pool(name="dramp", bufs=2, space="DRAM") as dramp,
        ):
            # ---- load constants / weights into SBUF ----
            w_e0 = wpool.tile([128, KB_E * G], FP, tag="w_e0")
            w_o0 = wpool.tile([128, KB_H * G], FP, tag="w_o0")
            w_h0 = wpool.tile([128, KB_H * G], FP, tag="w_h0")
            w_i1 = wpool.tile([128, KB_H * G], FP, tag="w_i1")
            w_h1 = wpool.tile([128, KB_H * G], FP, tag="w_h1")
            bias0 = wpool.tile([1, G], FP, tag="bias0")
            bias1 = wpool.tile([1, G], FP, tag="bias1")
            ones = wpool.tile([1, B], FP, tag="ones")
            ident = wpool.tile([B, B], FP, tag="ident")
            outT_init = wpool.tile([128, NC * B], FP, tag="outTi")
            h0T_init = wpool.tile([128, NC * B], FP, tag="h0Ti")
            h1T_init = wpool.tile([128, NC * B], FP, tag="h1Ti")

            for sb, dr in [
                (w_e0, w_e0_d), (w_o0, w_o0_d), (w_h0, w_h0_d),
                (w_i1, w_i1_d), (w_h1, w_h1_d),
                (bias0, bias0_d), (bias1, bias1_d),
                (ones, ones_d), (ident, ident_d),
                (outT_init, outTi_d), (h0T_init, h0Ti_d), (h1T_init, h1Ti_d),
            ]:
                nc.sync.dma_start(sb[:], dr[:])

            c0_prev = sp.tile([B, HC], FP, tag="c0")
            c1_prev = sp.tile([B, HC], FP, tag="c1")
            nc.sync.dma_start(c0_prev[:], c0i_d[:])
            nc.sync.dma_start(c1_prev[:], c1i_d[:])

            outT_prev = outT_init  # gathered out^T   [128, 8*64]
            h0T_prev = h0T_init    # gathered h0^T    [128, 8*64]
            h1T_prev = h1T_init    # gathered h1^T    [128, 8*64]

            if comm == "remote":
                rsems = [nc.alloc_semaphore("rsem0"), nc.alloc_semaphore("rsem1")]
                lsems = [nc.alloc_semaphore("lsem0"), nc.alloc_semaphore("lsem1")]
                # remote_sem inc per sender per gather: 16 // 8 dests = 2
                RS_INC_PER_STEP = 2 * (NC - 1)  # 14

            def gather_collective(ptti, tagp, si, t):
                """All-gather this core's h^T chunk [128,64] -> [128, 8*64]."""
                pt, _ti = ptti
                hbt = sp.tile([HC, B], FP, tag=f"hbt{tagp}")
                nc.vector.tensor_copy(hbt[:], pt[:])
                agin = dramp.tile([128, B], FP, tag=f"agin{tagp}")
                agout = dramp.tile([128 * NC, B], FP, tag=f"agout{tagp}")
                nc.sync.dma_start(agin[:], hbt[:])
                nc.gpsimd.collective_compute(
                    "AllGather",
                    mybir.AluOpType.bypass,
                    ins=[agin.opt()],
                    outs=[agout.opt()],
                    replica_groups=[list(range(NC))],
                )
                gt = gp.tile([128, NC * B], FP, tag=f"g{tagp}")
                nc.sync.dma_start(
                    gt[:].rearrange("p (b n) -> p b n", n=B),
                    agout[:].rearrange("(b p) n -> p b n", p=128),
                )
                return gt, None

            def gather_remote(ptti, tagp, si, t):
                """One-hop mesh all-gather: write own chunk into slot 0, then
                remote-DMA it into slot d of each XOR peer (me^d)."""
                pt, ti = ptti
                gt = gp.tile([128, NC * B], FP, tag=f"g{tagp}")
                if t >= 2:
                    # WAR: this pool slot was last used by gather t-2 whose
                    # sends read slot 0 asynchronously after their trigger --
                    # retired only by the local send-completion sem (16 per
                    # broadcast, 7 per gather). Wait for ALL sends through
                    # gather t-1 (unambiguous milestone at this point).
                    dn = nc.vector.nop(nofuse=True, hint=f"lsem_gate{si}")
                    add_dep_helper(dn.ins, ti.ins, False, "lsem gate anchor")
                    deferred_waits.append((dn, lsems[si], 16 * (NC - 1) * t))
                cp = nc.vector.tensor_copy(gt[:, 0:B], pt[:])
                for d in range(1, NC):
                    rdests: list = [None] * NC
                    rdests[d] = (0, d)
                    nc.gpsimd.remote_dma_broadcast(
                        out_ap=gt[:, d * B : (d + 1) * B],
                        in_ap=gt[:, 0:B],
                        remote_sem=rsems[si],
                        local_sem=lsems[si],
                        rdests=rdests,
                        queue_num=si,
                    )
                nc.gpsimd.trigger_dma(count=None, queue_num=si)
                return gt, cp

            gather = gather_remote if comm == "remote" else gather_collective
            g0_cp = g1_cp = None

            def lstm_elem(pg, c_prev, tag):
                """Gate nonlinearities + cell update. pg: PSUM [64, 512] in
                [i|f|o|g] order. Returns (h_chunk [64,128], c_new [64,128])."""
                sg = sp.tile([B, 3 * HC], FP, tag=f"sg{tag}")
                tg = sp.tile([B, HC], FP, tag=f"tg{tag}")
                nc.scalar.activation(sg[:], pg[:, 0 : 3 * HC], AF.Sigmoid)
                nc.scalar.activation(tg[:], pg[:, 3 * HC : 4 * HC], AF.Tanh)
                m1 = sp.tile([B, HC], FP, tag=f"m1{tag}")
                m2 = sp.tile([B, HC], FP, tag=f"m2{tag}")
                c_new = sp.tile([B, HC], FP, tag=f"c{tag}")
                nc.vector.tensor_mul(m1[:], sg[:, HC : 2 * HC], c_prev[:])
                nc.vector.tensor_mul(m2[:], sg[:, 0:HC], tg[:])
                nc.vector.tensor_add(c_new[:], m1[:], m2[:])
                tc_ = sp.tile([B, HC], FP, tag=f"tc{tag}")
                nc.scalar.activation(tc_[:], c_new[:], AF.Tanh)
                h_chunk = sp.tile([B, HC], FP, tag=f"h{tag}")
                nc.vector.tensor_mul(h_chunk[:], sg[:, 2 * HC : 3 * HC], tc_[:])
                return h_chunk, c_new

            def transpose_chunk(h_chunk, tag):
                """[64,128] batch-major -> [128,64] feature-major (PSUM)."""
                pt = pst.tile([HC, B], FP, tag="pt")  # shared slots: 2 banks
                ti = nc.tensor.transpose(pt[:], h_chunk[:], ident[:])
                return pt, ti

            for t in range(s_steps):
                # ---------- layer 0 gates: [64, 512] in PSUM ----------
                pg0 = psg.tile([B, G], FP, tag="pg0")
                embt = embp.tile([128, KB_E * B], FP, tag="embt")
                nc.sync.dma_start(embt[:], embT_d[t, :, :])
                for k in range(KB_E):  # embedding part (prefetchable)
                    nc.tensor.matmul(
                        pg0[:],
                        embt[:, k * B : (k + 1) * B],
                        w_e0[:, k * G : (k + 1) * G],
                        start=(k == 0),
                        stop=False,
                    )
                nc.tensor.matmul(  # + bias0 (ones ⊗ bias row)
                    pg0[:], ones[:1, :], bias0[:1, :], start=False, stop=False
                )
                for k in range(KB_H):  # recurrent part (h0_{t-1})
                    nc.tensor.matmul(
                        pg0[:],
                        h0T_prev[:, k * B : (k + 1) * B],
                        w_h0[:, k * G : (k + 1) * G],
                        start=False,
                        stop=False,
                    )
                if comm == "remote" and t > 0:
                    # out_{t-1} = h1_{t-1}: wait for step t-1's h1 gather.
                    # Anchor on our own slot-0 copy so the scheduler places
                    # the wait right before the consuming matmuls.
                    wn = nc.tensor.nop(nofuse=True, hint="wait_g1")
                    add_dep_helper(wn.ins, g1_cp.ins, False, "g1 wait anchor")
                    deferred_waits.append((wn, rsems[1], RS_INC_PER_STEP * t))
                for k in range(KB_H):  # input-feed part (out_{t-1})
                    nc.tensor.matmul(
                        pg0[:],
                        outT_prev[:, k * B : (k + 1) * B],
                        w_o0[:, k * G : (k + 1) * G],
                        start=False,
                        stop=(k == KB_H - 1),
                    )

                h0_chunk, c0_new = lstm_elem(pg0, c0_prev, "0")
                g0, g0_cp = gather(transpose_chunk(h0_chunk, "0"), "0", 0, t)

                # ---------- layer 1 gates ----------
                pg1 = psg.tile([B, G], FP, tag="pg1")
                nc.tensor.matmul(  # bias1 first (starts accumulation)
                    pg1[:], ones[:1, :], bias1[:1, :], start=True, stop=False
                )
                for k in range(KB_H):  # recurrent part (h1_{t-1})
                    nc.tensor.matmul(
                        pg1[:],
                        h1T_prev[:, k * B : (k + 1) * B],
                        w_h1[:, k * G : (k + 1) * G],
                        start=False,
                        stop=False,
                    )
                if comm == "remote":
                    wn = nc.tensor.nop(nofuse=True, hint="wait_g0")
                    add_dep_helper(wn.ins, g0_cp.ins, False, "g0 wait anchor")
                    deferred_waits.append((wn, rsems[0], RS_INC_PER_STEP * (t + 1)))
                for k in range(KB_H):  # input part (h0_t, just gathered)
                    nc.tensor.matmul(
                        pg1[:],
                        g0[:, k * B : (k + 1) * B],
                        w_i1[:, k * G : (k + 1) * G],
                        start=False,
                        stop=(k == KB_H - 1),
                    )

                h1_chunk, c1_new = lstm_elem(pg1, c1_prev, "1")
                nc.sync.dma_start(outs_d[t, :, :], h1_chunk[:])
                g1, g1_cp = gather(transpose_chunk(h1_chunk, "1"), "1", 1, t)

                # rotate state
                h0T_prev = g0
                h1T_prev = g1
                outT_prev = g1
                c0_prev = c0_new
                c1_prev = c1_new

                if t == s_steps - 1:
                    nc.sync.dma_start(hn_d[0, :, :], h0_chunk[:])
                    nc.sync.dma_start(hn_d[1, :, :], h1_chunk[:])
                    nc.sync.dma_start(cn_d[0, :, :], c0_new[:])
                    nc.sync.dma_start(cn_d[1, :, :], c1_new[:])

    for mm, sem, val in deferred_waits:
        mm._wait_ge(sem, val)
    nc.compile()
    return nc


# ---------------------------------------------------------------------------
# Host-side data prep
# ---------------------------------------------------------------------------

def _gate_rows(j: int) -> np.ndarray:
    """Rows of the [4H, *] weight matrices owned by core j, in [i,f,o,g] order."""
    base = np.arange(j * HC, (j + 1) * HC)
    return np.concatenate([base, H + base, 3 * H + base, 2 * H + base])


def _swizzle_kblocks(wt: np.ndarray) -> np.ndarray:
    """[K, G] (K = n*128) -> [128, n*G] with K-block b at free cols [b*G:(b+1)*G]."""
    k, g = wt.shape
    nb = k // 128
    return np.ascontiguousarray(
        wt.reshape(nb, 128, g).transpose(1, 0, 2).reshape(128, nb * g)
    )


def _gatherT(full: np.ndarray, perm) -> np.ndarray:
    """[B, H] state -> gathered-transposed [128, 8*64]: slot p = chunk perm[p]^T."""
    out = np.empty((128, NC * B), dtype=np.float32)
    for p in range(NC):
        blk = full[:, perm[p] * HC : (perm[p] + 1) * HC]  # [B, HC]
        out[:, p * B : (p + 1) * B] = blk.T
    return np.ascontiguousarray(out)


def _prep_inputs(inputs: dict, s_steps: int, comm: str = "collective"):
    """Build per-core in_maps.

    K-block permutation: collective gathers land in natural rank order
    (perm[j][p] = p); the remote_dma mesh lands in XOR slot order
    (slot p on core j = chunk of core j^p), so weights/init states are
    permuted to match."""
    tgt = np.asarray(inputs["tgt_input"])[:s_steps]
    emb_W = np.asarray(inputs["emb_W"], dtype=np.float32)
    h0 = np.asarray(inputs["h0"], dtype=np.float32)
    c0 = np.asarray(inputs["c0"], dtype=np.float32)
    init_out = np.asarray(inputs["init_output"], dtype=np.float32)
    W_ih0 = np.asarray(inputs["W_ih0"], dtype=np.float32)
    W_hh0 = np.asarray(inputs["W_hh0"], dtype=np.float32)
    W_ih1 = np.asarray(inputs["W_ih1"], dtype=np.float32)
    W_hh1 = np.asarray(inputs["W_hh1"], dtype=np.float32)
    b0 = np.asarray(inputs["b_ih0"], dtype=np.float32) + np.asarray(
        inputs["b_hh0"], dtype=np.float32
    )
    b1 = np.asarray(inputs["b_ih1"], dtype=np.float32) + np.asarray(
        inputs["b_hh1"], dtype=np.float32
    )

    # embedding gather + transpose + K-block swizzle: [S, 128, 4*64]
    embs = emb_W[tgt]  # [S, B, E]
    embT = np.ascontiguousarray(
        embs.transpose(0, 2, 1)  # [S, E, B]
        .reshape(s_steps, KB_E, 128, B)
        .transpose(0, 2, 1, 3)
        .reshape(s_steps, 128, KB_E * B)
    ).astype(np.float32)

    ones = np.ones((1, B), dtype=np.float32)
    ident = np.eye(B, dtype=np.float32)

    in_maps = []
    for j in range(NC):
        rows = _gate_rows(j)
        if comm == "remote":
            perm = [j ^ p for p in range(NC)]
        else:
            perm = list(range(NC))
        hperm = np.concatenate(
            [np.arange(perm[p] * HC, (perm[p] + 1) * HC) for p in range(NC)]
        )
        w_e0 = _swizzle_kblocks(W_ih0[rows, :E].T)  # [128, 4*512]
        w_o0 = _swizzle_kblocks(W_ih0[rows, E:].T[hperm])  # [128, 8*512]
        w_h0 = _swizzle_kblocks(W_hh0[rows].T[hperm])
        w_i1 = _swizzle_kblocks(W_ih1[rows].T[hperm])
        w_h1 = _swizzle_kblocks(W_hh1[rows].T[hperm])
        in_maps.append(
            {
                "embT": embT,
                "w_e0": w_e0,
                "w_o0": w_o0,
                "w_h0": w_h0,
                "w_i1": w_i1,
                "w_h1": w_h1,
                "bias0": b0[rows][None, :].astype(np.float32),
                "bias1": b1[rows][None, :].astype(np.float32),
                "ones": ones,
                "ident": ident,
                "outT_init": _gatherT(init_out, perm),
                "h0T_init": _gatherT(h0[0], perm),
                "h1T_init": _gatherT(h0[1], perm),
                "c0_init": np.ascontiguousarray(c0[0][:, j * HC : (j + 1) * HC]),
                "c1_init": np.ascontiguousarray(c0[1][:, j * HC : (j + 1) * HC]),
            }
        )
    return in_maps


def _assemble(results, s_steps: int):
    outs = [np.asarray(r["outs"]).reshape(s_steps, B, HC) for r in results]
    hns = [np.asarray(r["hn"]).reshape(2, B, HC) for r in results]
    cns = [np.asarray(r["cn"]).reshape(2, B, HC) for r in results]
    outputs = np.concatenate(outs, axis=2)  # [S, B, H]
    h_n = np.concatenate(hns, axis=2)  # [2, B, H]
    c_n = np.concatenate(cns, axis=2)  # [2, B, H]
    return outputs, h_n, c_n


_NC_CACHE: dict = {}

COMM = "collective"  # module-level default comm mode


def _get_nc(s_steps: int, comm: str):
    key = (s_steps, comm)
    if key not in _NC_CACHE:
        _NC_CACHE[key] = _build_nc(s_steps, comm)
    return _NC_CACHE[key]


def kernel(
    tgt_input,
    h0,
    c0,
    init_output,
    emb_W,
    W_ih0,
    W_hh0,
    b_ih0,
    b_hh0,
    W_ih1,
    W_hh1,
    b_ih1,
    b_hh1,
    _s_steps: int = S,
    _trace: bool = False,
):
    inputs = dict(
        tgt_input=tgt_input,
        h0=h0,
        c0=c0,
        init_output=init_output,
        emb_W=emb_W,
        W_ih0=W_ih0,
        W_hh0=W_hh0,
        b_ih0=b_ih0,
        b_hh0=b_hh0,
        W_ih1=W_ih1,
        W_hh1=W_hh1,
        b_ih1=b_ih1,
        b_hh1=b_hh1,
    )
    nc = _get_nc(_s_steps, COMM)
    in_maps = _prep_inputs(inputs, _s_steps, COMM)
    res = run_bass_kernel_spmd(
        nc, in_maps, core_ids=list(range(NC)), trace=_trace
    )
    out = _assemble(res.results, _s_steps)
    if _trace:
        return out, res
    return out
